# revision 22
# baseline (speedup 1.0000x reference)
"""CustomRNN (Elman cell) Trainium2 kernel.

Problem: x [T=512, B=64, D=1024], W_ih/W_hh [1024,1024], b_ih/b_hh [1024].
  xproj = einsum('tbd,hd->tbh', x, W_ih) + b_ih
  h_t = tanh(xproj[t] + h_{t-1} @ W_hh.T + b_hh),  h_0 = 0
  returns (output [T,B,D] = all h_t, h_final [B,D])

Strategy: data-parallel over batch B across 8 cores (8 samples/core).
Each core:
  Phase 1: xproj GEMM (with both biases folded in via an identity-matmul
           accumulation) -> DRAM scratch, natural [t*8+b, d] layout.
  Phase 2: 512 sequential steps. State kept transposed (hT [128, 64] in
           SBUF, chunk c at cols 8c:8c+8 holding h[b, 128c+p]).
           Per step: pre = u_t (identity matmul) + sum_c hT_c.T @ W_hhT_c
           accumulated in PSUM [8, 1024] (2 banks), tanh on ACT -> h [8,1024],
           DMA h to y, PE-transpose h back to hT for the next step.
All matmuls use K=128 contraction. Weights pre-transposed on host.

Walrus constraint baked into the structure: one sync wait per DMA
instruction, two per compute instruction. One-time loads funnel through a
DVE copy; persistent buffers are allocated before phase 1 so no SBUF
address reuse creates cross-engine WAR deps.
"""

import os
import sys

import numpy as np

for _p in ("/opt/trn_rl_repo", "/root/.axon_site/_ro/trn_rl_repo"):
    if os.path.isdir(_p) and _p not in sys.path:
        sys.path.append(_p)

import concourse.bass as bass
import concourse.mybir as mybir
import concourse.tile as tile
from concourse import bacc
from concourse.bass import ds
from concourse.bass_utils import run_bass_kernel_spmd

T_TOTAL = 512
B_TOTAL = 64
D = 1024
N_CORES = 8
B_LOC = B_TOTAL // N_CORES          # 8 samples per core
KT = D // 128                       # 8 contraction chunks
TBLK = 16                           # timesteps per xproj tile (16*8 = 128 rows)
U = 32                              # unrolled steps per For_i iteration

FP32 = mybir.dt.float32
FP16 = mybir.dt.float16


def build_program(t_total=T_TOTAL):
    u_steps = min(U, t_total)
    nblk = t_total * B_LOC // 128
    nc = bacc.Bacc("TRN2", target_bir_lowering=False, debug=False)

    xT_in = nc.declare_dram_parameter("xT", [nblk, 128, KT * 128], FP16, isOutput=False)
    wihT_in = nc.declare_dram_parameter("wihT", [128, KT, D], FP16, isOutput=False)
    whhT_in = nc.declare_dram_parameter("whhT", [128, KT, D], FP16, isOutput=False)
    bb_in = nc.declare_dram_parameter("bb", [128, D], FP16, isOutput=False)
    id_in = nc.declare_dram_parameter("ident", [128, 128], FP32, isOutput=False)
    idh_in = nc.declare_dram_parameter("identh", [128, 128], FP16, isOutput=False)
    y_out = nc.declare_dram_parameter("y", [t_total * B_LOC, D], FP32, isOutput=True)

    xp_dram = nc.dram_tensor("xp_scratch", [t_total * B_LOC, D], FP16)

    with tile.TileContext(nc) as tc:
        with tc.tile_pool(name="persist", bufs=1) as persist:
            ident = persist.tile([128, 128], FP32)
            identh = persist.tile([128, 128], FP16)
            whhT = persist.tile([128, KT, D], FP16)
            hT = [persist.tile([128, 32 * KT], FP16, tag=f"hT{i}",
                               name=f"hT{i}") for i in range(2)]
            # u/h buffers are persistent and rotated manually so the
            # never-DMA'd rows 8..127 (read by K=128 matmuls against
            # zero lanes of the selection matrix) are memset exactly once.
            u_bufs = [persist.tile([128, D], FP16, tag=f"u{j}",
                                   name=f"u{j}") for j in range(4)]
            h_bufs = [persist.tile([128, D], FP16, tag=f"hb{j}",
                                   name=f"hb{j}") for j in range(2)]
            h32_bufs = [persist.tile([B_LOC, D], FP32, tag=f"h32{j}",
                                     name=f"h32{j}") for j in range(2)]
            nc.vector.memset(hT[0][:], 0.0)
            for b in u_bufs + h_bufs + h32_bufs:
                nc.vector.memset(b[:], 0.0)

            # ---------------- Phase 1: xproj = x @ W_ih.T + (b_ih + b_hh) ----
            with (
                tc.tile_pool(name="p1sb", bufs=3) as p1sb,
                tc.tile_pool(name="p1w", bufs=1) as p1w,
                tc.tile_pool(name="p1ps", bufs=2, space="PSUM") as p1ps,
            ):
                wihT = p1w.tile([128, KT, D], FP16)
                bb = p1w.tile([128, D], FP16)
                nc.sync.dma_start(whhT[:], whhT_in[:])
                nc.sync.dma_start(wihT[:], wihT_in[:])
                # ident/bb funnel through a DVE copy so the K-accumulation's
                # first matmul (which also pays the PSUM-release wait) depends
                # on the single DVE semaphore, not per-queue DMA sems.
                with tc.tile_pool(name="stage", bufs=1) as stage_pool:
                    for dst, src, tg in ((ident, id_in, "sm"), (bb, bb_in, "sb"),
                                         (identh, idh_in, "sh")):
                        stage = stage_pool.tile(list(dst.shape), dst.dtype,
                                                tag=tg, name="stage")
                        nc.sync.dma_start(stage[:], src[:])
                        nc.vector.tensor_copy(dst[:], stage[:])

                for blk in range(nblk):
                    xt = p1sb.tile([128, KT, 128], FP16, tag="xt")
                    nc.sync.dma_start(xt[:],
                                      xT_in[blk].rearrange("p (k f) -> p k f", k=KT))
                    xp = p1sb.tile([128, D], FP16, tag="xp")
                    for h in range(2):
                        psum = p1ps.tile([128, 512], FP32, tag=f"ps{h}")
                        cols = slice(512 * h, 512 * h + 512)
                        nc.tensor.matmul(psum[:], identh[:], bb[:, cols],
                                         start=True, stop=False)
                        for k in range(KT):
                            nc.tensor.matmul(psum[:], xt[:, k, :], wihT[:, k, cols],
                                             start=False, stop=(k == KT - 1))
                        nc.vector.tensor_copy(xp[:, cols], psum[:])
                    nc.sync.dma_start(xp_dram[blk * 128:(blk + 1) * 128, :], xp[:])

            # ---------------- Phase 2: sequential recurrence ----------------
            with tc.tile_pool(name="p2ps", bufs=2, space="PSUM") as p2ps:
                n_iter = t_total // u_steps

                def step_body(iv, u):
                    src = hT[u % 2]
                    dst = hT[1 - (u % 2)]
                    row0 = iv * (u_steps * B_LOC) + u * B_LOC

                    u_t = u_bufs[u % 4]
                    # gpsimd (SWDGE) for the dynamic load: keeps the SP
                    # engine's bounds-check register budget for the y store.
                    nc.gpsimd.dma_start(u_t[0:B_LOC, :], xp_dram[ds(row0, B_LOC), :])

                    ps = [p2ps.tile([B_LOC, 512], FP32, tag=f"pre{h}",
                                    name=f"pre{h}") for h in range(2)]
                    for h in range(2):
                        cols = slice(512 * h, 512 * h + 512)
                        nc.tensor.matmul(ps[h][:], identh[:, 0:B_LOC], u_t[:, cols],
                                         start=True, stop=False)
                        for c in range(KT):
                            nc.tensor.matmul(ps[h][:],
                                             src[:, c * 32:c * 32 + B_LOC],
                                             whhT[:, c, cols],
                                             start=False, stop=(c == KT - 1))

                    # tanh -> h16 in permuted column order (r, c, w):
                    # h16[b, r*256 + c*32 + w] = tanh(pre)[b, c*128 + r*32 + w]
                    # so each 32x32 DVE block transpose lands hT chunks at
                    # partition p = d % 128 with chunk c at cols 32c..32c+8.
                    h16 = h_bufs[u % 2]
                    h32 = h32_bufs[u % 2]
                    for h in range(2):
                        ps3 = ps[h].rearrange("b (cc rw) -> b cc rw", rw=128)
                        h16r = h16.rearrange("p (r q) -> p r q", r=4)
                        for r in range(4):
                            in_ap = ps3[:, :, r * 32:(r + 1) * 32]
                            out_ap = h16r[0:B_LOC, r,
                                          128 * h:128 * h + 128].rearrange(
                                              "b (cc w) -> b cc w", w=32)
                            nc.scalar.activation(out_ap, in_ap,
                                                 mybir.ActivationFunctionType.Tanh)
                    for r in range(4):
                        nc.vector.transpose(dst[r * 32:(r + 1) * 32, :],
                                            h16[0:32, r * 256:(r + 1) * 256])
                    # y wants fp32 in natural order; off the critical path
                    h32r = h32.rearrange("b (c rw) -> b c rw", rw=128)
                    for r in range(4):
                        nc.vector.tensor_copy(
                            h32r[:, :, r * 32:(r + 1) * 32],
                            h16[0:B_LOC, r * 256:(r + 1) * 256].rearrange(
                                "b (c w) -> b c w", w=32))
                    nc.sync.dma_start(y_out[ds(row0, B_LOC), :], h32[:])

                with tc.For_i(0, n_iter, 1, staggered_reset=True,
                              hint_engines=(mybir.EngineType.PE,)) as iv:
                    for u in range(u_steps):
                        step_body(iv, u)

    nc.compile()
    return nc


def host_prep(x_shard, t_total=T_TOTAL):
    """Build the per-core input map from a [t, B_LOC, D] x shard."""
    nblk = t_total * B_LOC // 128
    xr = np.ascontiguousarray(x_shard).reshape(nblk, TBLK, B_LOC, KT, 128)
    # -> [blk, p, k, tl, b] -> [blk, 128, KT*128]
    xT = np.ascontiguousarray(
        xr.transpose(0, 4, 3, 1, 2).astype(np.float16)
    ).reshape(nblk, 128, KT * 128)
    return {"xT": xT}


def _shared_inputs(W_ih, W_hh, b_ih, b_hh):
    wihT = np.ascontiguousarray(W_ih.T).reshape(KT, 128, D).transpose(1, 0, 2)
    whhT = np.ascontiguousarray(W_hh.T).reshape(KT, 128, D).transpose(1, 0, 2)
    bb = np.tile((b_ih + b_hh).astype(np.float16)[None, :], (128, 1))
    return {
        "wihT": np.ascontiguousarray(wihT, dtype=np.float16),
        "whhT": np.ascontiguousarray(whhT, dtype=np.float16),
        "bb": bb,
        "ident": np.eye(128, dtype=np.float32),
        "identh": np.eye(128, dtype=np.float16),
    }


_NC_CACHE = {}


def _get_program(t_total):
    if t_total not in _NC_CACHE:
        _NC_CACHE[t_total] = build_program(t_total)
    return _NC_CACHE[t_total]


def kernel(x, W_ih, W_hh, b_ih, b_hh, _t_total=None, _trace=False):
    x = np.asarray(x, dtype=np.float32)
    W_ih = np.asarray(W_ih, dtype=np.float32)
    W_hh = np.asarray(W_hh, dtype=np.float32)
    b_ih = np.asarray(b_ih, dtype=np.float32)
    b_hh = np.asarray(b_hh, dtype=np.float32)

    t_total = _t_total or x.shape[0]
    nc = _get_program(t_total)
    shared = _shared_inputs(W_ih, W_hh, b_ih, b_hh)

    in_maps = []
    for c in range(N_CORES):
        shard = x[:, c * B_LOC:(c + 1) * B_LOC, :]
        m = dict(shared)
        m.update(host_prep(shard, t_total))
        in_maps.append(m)

    res = run_bass_kernel_spmd(nc, in_maps, list(range(N_CORES)), trace=_trace)
    ys = [res.results[c]["y"].reshape(t_total, B_LOC, D) for c in range(N_CORES)]
    y = np.concatenate(ys, axis=1)
    out = (y, y[-1].copy())
    if _trace:
        return out, res
    return out


# revision 23
# speedup vs baseline: 1.1865x; 1.1865x over previous
"""CustomRNN (Elman cell) Trainium2 kernel.

Problem: x [T=512, B=64, D=1024], W_ih/W_hh [1024,1024], b_ih/b_hh [1024].
  xproj = einsum('tbd,hd->tbh', x, W_ih) + b_ih
  h_t = tanh(xproj[t] + h_{t-1} @ W_hh.T + b_hh),  h_0 = 0
  returns (output [T,B,D] = all h_t, h_final [B,D])

Strategy: data-parallel over batch B across 8 cores (8 samples/core).
Each core:
  Phase 1: xproj GEMM (with both biases folded in via an identity-matmul
           accumulation) -> DRAM scratch, natural [t*8+b, d] layout.
  Phase 2: 512 sequential steps. State kept transposed (hT [128, 64] in
           SBUF, chunk c at cols 8c:8c+8 holding h[b, 128c+p]).
           Per step: pre = u_t (identity matmul) + sum_c hT_c.T @ W_hhT_c
           accumulated in PSUM [8, 1024] (2 banks), tanh on ACT -> h [8,1024],
           DMA h to y, PE-transpose h back to hT for the next step.
All matmuls use K=128 contraction. Weights pre-transposed on host.

Walrus constraint baked into the structure: one sync wait per DMA
instruction, two per compute instruction. One-time loads funnel through a
DVE copy; persistent buffers are allocated before phase 1 so no SBUF
address reuse creates cross-engine WAR deps.
"""

import os
import sys

import numpy as np

for _p in ("/opt/trn_rl_repo", "/root/.axon_site/_ro/trn_rl_repo"):
    if os.path.isdir(_p) and _p not in sys.path:
        sys.path.append(_p)

import concourse.bass as bass
import concourse.mybir as mybir
import concourse.tile as tile
from concourse import bacc
from concourse.bass import ds
from concourse.bass_utils import run_bass_kernel_spmd

T_TOTAL = 512
B_TOTAL = 64
D = 1024
N_CORES = 8
B_LOC = B_TOTAL // N_CORES          # 8 samples per core
KT = D // 128                       # 8 contraction chunks
TBLK = 16                           # timesteps per xproj tile (16*8 = 128 rows)
U = 16                              # unrolled steps per For_i iteration

FP32 = mybir.dt.float32
FP16 = mybir.dt.float16


def build_program(t_total=T_TOTAL):
    nblk = t_total * B_LOC // 128
    nc = bacc.Bacc("TRN2", target_bir_lowering=False, debug=False)

    xT_in = nc.declare_dram_parameter("xT", [nblk, 128, KT * 128], FP16, isOutput=False)
    wihT_in = nc.declare_dram_parameter("wihT", [128, KT, D], FP16, isOutput=False)
    whhT_in = nc.declare_dram_parameter("whhT", [128, KT, D], FP16, isOutput=False)
    bb_in = nc.declare_dram_parameter("bb", [128, D], FP16, isOutput=False)
    id_in = nc.declare_dram_parameter("ident", [128, 128], FP32, isOutput=False)
    idh_in = nc.declare_dram_parameter("identh", [128, 128], FP16, isOutput=False)
    y_out = nc.declare_dram_parameter("y", [t_total * B_LOC, D], FP32, isOutput=True)

    xp_dram = nc.dram_tensor("xp_scratch", [t_total * B_LOC, D], FP16)

    with tile.TileContext(nc) as tc:
        with tc.tile_pool(name="persist", bufs=1) as persist:
            ident = persist.tile([128, 128], FP32)
            identh = persist.tile([128, 128], FP16)
            whhT = persist.tile([128, KT, D], FP16)
            hT = [persist.tile([128, B_LOC * KT], FP16, tag=f"hT{i}",
                               name=f"hT{i}") for i in range(2)]
            # u/h buffers are persistent and rotated manually so the
            # never-DMA'd rows 8..127 (read by K=128 matmuls against
            # zero lanes of the selection matrix) are memset exactly once.
            u_bufs = [persist.tile([128, D], FP16, tag=f"u{j}",
                                   name=f"u{j}") for j in range(4)]
            h_bufs = [persist.tile([128, D], FP16, tag=f"hb{j}",
                                   name=f"hb{j}") for j in range(2)]
            h32_bufs = [persist.tile([B_LOC, D], FP32, tag=f"h32{j}",
                                     name=f"h32{j}") for j in range(2)]
            nc.vector.memset(hT[0][:], 0.0)
            for b in u_bufs + h_bufs + h32_bufs:
                nc.vector.memset(b[:], 0.0)

            # ---------------- Phase 1: xproj = x @ W_ih.T + (b_ih + b_hh) ----
            with (
                tc.tile_pool(name="p1sb", bufs=3) as p1sb,
                tc.tile_pool(name="p1w", bufs=1) as p1w,
                tc.tile_pool(name="p1ps", bufs=2, space="PSUM") as p1ps,
            ):
                wihT = p1w.tile([128, KT, D], FP16)
                bb = p1w.tile([128, D], FP16)
                nc.sync.dma_start(whhT[:], whhT_in[:])
                nc.sync.dma_start(wihT[:], wihT_in[:])
                # ident/bb funnel through a DVE copy so the K-accumulation's
                # first matmul (which also pays the PSUM-release wait) depends
                # on the single DVE semaphore, not per-queue DMA sems.
                with tc.tile_pool(name="stage", bufs=1) as stage_pool:
                    for dst, src, tg in ((ident, id_in, "sm"), (bb, bb_in, "sb"),
                                         (identh, idh_in, "sh")):
                        stage = stage_pool.tile(list(dst.shape), dst.dtype,
                                                tag=tg, name="stage")
                        nc.sync.dma_start(stage[:], src[:])
                        nc.vector.tensor_copy(dst[:], stage[:])

                for blk in range(nblk):
                    xt = p1sb.tile([128, KT, 128], FP16, tag="xt")
                    nc.sync.dma_start(xt[:],
                                      xT_in[blk].rearrange("p (k f) -> p k f", k=KT))
                    xp = p1sb.tile([128, D], FP16, tag="xp")
                    for h in range(2):
                        psum = p1ps.tile([128, 512], FP32, tag=f"ps{h}")
                        cols = slice(512 * h, 512 * h + 512)
                        nc.tensor.matmul(psum[:], identh[:], bb[:, cols],
                                         start=True, stop=False)
                        for k in range(KT):
                            nc.tensor.matmul(psum[:], xt[:, k, :], wihT[:, k, cols],
                                             start=False, stop=(k == KT - 1))
                        nc.vector.tensor_copy(xp[:, cols], psum[:])
                    nc.sync.dma_start(xp_dram[blk * 128:(blk + 1) * 128, :], xp[:])

            # ---------------- Phase 2: sequential recurrence ----------------
            with tc.tile_pool(name="p2ps", bufs=2, space="PSUM") as p2ps:
                n_iter = t_total // U

                def step_body(iv, u):
                    src = hT[u % 2]
                    dst = hT[1 - (u % 2)]
                    row0 = iv * (U * B_LOC) + u * B_LOC

                    u_t = u_bufs[u % 4]
                    # gpsimd (SWDGE) for the dynamic load: keeps the SP
                    # engine's bounds-check register budget for the y store.
                    nc.gpsimd.dma_start(u_t[0:B_LOC, :], xp_dram[ds(row0, B_LOC), :])

                    ps = [p2ps.tile([B_LOC, 512], FP32, tag=f"pre{h}",
                                    name=f"pre{h}") for h in range(2)]
                    for h in range(2):
                        cols = slice(512 * h, 512 * h + 512)
                        nc.tensor.matmul(ps[h][:], identh[:, 0:B_LOC], u_t[:, cols],
                                         start=True, stop=False)
                        for c in range(KT):
                            nc.tensor.matmul(ps[h][:],
                                             src[:, c * B_LOC:(c + 1) * B_LOC],
                                             whhT[:, c, cols],
                                             start=False, stop=(c == KT - 1))

                    h_sb = h_bufs[u % 2]
                    h32 = h32_bufs[u % 2]
                    for q in range(4):
                        cols = slice(256 * q, 256 * q + 256)
                        nc.scalar.activation(h_sb[0:B_LOC, cols],
                                             ps[q // 2][:, 256 * (q % 2):
                                                        256 * (q % 2) + 256],
                                             mybir.ActivationFunctionType.Tanh)
                    # y wants fp32; the up-cast runs off the critical path
                    nc.vector.tensor_copy(h32[:], h_sb[0:B_LOC, :])
                    nc.sync.dma_start(y_out[ds(row0, B_LOC), :], h32[:])

                    for c in range(KT):
                        tp = p2ps.tile([128, B_LOC], FP16, tag="tp")
                        nc.tensor.transpose(tp[:],
                                            h_sb[0:B_LOC, c * 128:(c + 1) * 128],
                                            identh[0:B_LOC, 0:B_LOC])
                        nc.vector.tensor_copy(dst[:, c * B_LOC:(c + 1) * B_LOC],
                                              tp[:])

                with tc.For_i(0, n_iter, 1, staggered_reset=True,
                              hint_engines=(mybir.EngineType.PE,)) as iv:
                    for u in range(U):
                        step_body(iv, u)

    nc.compile()
    return nc


def host_prep(x_shard, t_total=T_TOTAL):
    """Build the per-core input map from a [t, B_LOC, D] x shard."""
    nblk = t_total * B_LOC // 128
    xr = np.ascontiguousarray(x_shard).reshape(nblk, TBLK, B_LOC, KT, 128)
    # -> [blk, p, k, tl, b] -> [blk, 128, KT*128]
    xT = np.ascontiguousarray(
        xr.transpose(0, 4, 3, 1, 2).astype(np.float16)
    ).reshape(nblk, 128, KT * 128)
    return {"xT": xT}


def _shared_inputs(W_ih, W_hh, b_ih, b_hh):
    wihT = np.ascontiguousarray(W_ih.T).reshape(KT, 128, D).transpose(1, 0, 2)
    whhT = np.ascontiguousarray(W_hh.T).reshape(KT, 128, D).transpose(1, 0, 2)
    bb = np.tile((b_ih + b_hh).astype(np.float16)[None, :], (128, 1))
    return {
        "wihT": np.ascontiguousarray(wihT, dtype=np.float16),
        "whhT": np.ascontiguousarray(whhT, dtype=np.float16),
        "bb": bb,
        "ident": np.eye(128, dtype=np.float32),
        "identh": np.eye(128, dtype=np.float16),
    }


_NC_CACHE = {}


def _get_program(t_total):
    if t_total not in _NC_CACHE:
        _NC_CACHE[t_total] = build_program(t_total)
    return _NC_CACHE[t_total]


def kernel(x, W_ih, W_hh, b_ih, b_hh, _t_total=None, _trace=False):
    x = np.asarray(x, dtype=np.float32)
    W_ih = np.asarray(W_ih, dtype=np.float32)
    W_hh = np.asarray(W_hh, dtype=np.float32)
    b_ih = np.asarray(b_ih, dtype=np.float32)
    b_hh = np.asarray(b_hh, dtype=np.float32)

    t_total = _t_total or x.shape[0]
    nc = _get_program(t_total)
    shared = _shared_inputs(W_ih, W_hh, b_ih, b_hh)

    in_maps = []
    for c in range(N_CORES):
        shard = x[:, c * B_LOC:(c + 1) * B_LOC, :]
        m = dict(shared)
        m.update(host_prep(shard, t_total))
        in_maps.append(m)

    res = run_bass_kernel_spmd(nc, in_maps, list(range(N_CORES)), trace=_trace)
    ys = [res.results[c]["y"].reshape(t_total, B_LOC, D) for c in range(N_CORES)]
    y = np.concatenate(ys, axis=1)
    out = (y, y[-1].copy())
    if _trace:
        return out, res
    return out


# revision 25
# speedup vs baseline: 1.1995x; 1.0109x over previous
"""CustomRNN (Elman cell) Trainium2 kernel.

Problem: x [T=512, B=64, D=1024], W_ih/W_hh [1024,1024], b_ih/b_hh [1024].
  xproj = einsum('tbd,hd->tbh', x, W_ih) + b_ih
  h_t = tanh(xproj[t] + h_{t-1} @ W_hh.T + b_hh),  h_0 = 0
  returns (output [T,B,D] = all h_t, h_final [B,D])

Strategy: data-parallel over batch B across 8 cores (8 samples/core).
Each core:
  Phase 1: xproj GEMM (with both biases folded in via an identity-matmul
           accumulation) -> DRAM scratch, natural [t*8+b, d] layout.
  Phase 2: 512 sequential steps. State kept transposed (hT [128, 64] in
           SBUF, chunk c at cols 8c:8c+8 holding h[b, 128c+p]).
           Per step: pre = u_t (identity matmul) + sum_c hT_c.T @ W_hhT_c
           accumulated in PSUM [8, 1024] (2 banks), tanh on ACT -> h [8,1024],
           DMA h to y, PE-transpose h back to hT for the next step.
All matmuls use K=128 contraction. Weights pre-transposed on host.

Walrus constraint baked into the structure: one sync wait per DMA
instruction, two per compute instruction. One-time loads funnel through a
DVE copy; persistent buffers are allocated before phase 1 so no SBUF
address reuse creates cross-engine WAR deps.
"""

import os
import sys

import numpy as np

for _p in ("/opt/trn_rl_repo", "/root/.axon_site/_ro/trn_rl_repo"):
    if os.path.isdir(_p) and _p not in sys.path:
        sys.path.append(_p)

import concourse.bass as bass
import concourse.mybir as mybir
import concourse.tile as tile
from concourse import bacc
from concourse.bass import ds
from concourse.bass_utils import run_bass_kernel_spmd

T_TOTAL = 512
B_TOTAL = 64
D = 1024
N_CORES = 8
B_LOC = B_TOTAL // N_CORES          # 8 samples per core
KT = D // 128                       # 8 contraction chunks
TBLK = 16                           # timesteps per xproj tile (16*8 = 128 rows)
U = 16                              # unrolled steps per For_i iteration

FP32 = mybir.dt.float32
FP16 = mybir.dt.float16


def build_program(t_total=T_TOTAL):
    nblk = t_total * B_LOC // 128
    nc = bacc.Bacc("TRN2", target_bir_lowering=False, debug=False)

    xT_in = nc.declare_dram_parameter("xT", [nblk, 128, KT * 128], FP16, isOutput=False)
    wihT_in = nc.declare_dram_parameter("wihT", [128, KT, D], FP16, isOutput=False)
    whhT_in = nc.declare_dram_parameter("whhT", [128, KT, D], FP16, isOutput=False)
    bb_in = nc.declare_dram_parameter("bb", [128, D], FP16, isOutput=False)
    id_in = nc.declare_dram_parameter("ident", [128, 128], FP32, isOutput=False)
    idh_in = nc.declare_dram_parameter("identh", [128, 128], FP16, isOutput=False)
    y_out = nc.declare_dram_parameter("y", [t_total * B_LOC, D], FP32, isOutput=True)

    xp_dram = nc.dram_tensor("xp_scratch", [t_total * B_LOC, D], FP16)

    with tile.TileContext(nc) as tc:
        with tc.tile_pool(name="persist", bufs=1) as persist:
            ident = persist.tile([128, 128], FP32)
            identh = persist.tile([128, 128], FP16)
            whhT = persist.tile([128, KT, D], FP16)
            hT = [persist.tile([128, B_LOC * KT], FP16, tag=f"hT{i}",
                               name=f"hT{i}") for i in range(2)]
            # u/h buffers are persistent and rotated manually so the
            # never-DMA'd rows 8..127 (read by K=128 matmuls against
            # zero lanes of the selection matrix) are memset exactly once.
            u_bufs = [persist.tile([128, D], FP16, tag=f"u{j}",
                                   name=f"u{j}") for j in range(4)]
            h_bufs = [persist.tile([128, D], FP16, tag=f"hb{j}",
                                   name=f"hb{j}") for j in range(2)]
            h32_bufs = [persist.tile([B_LOC, D], FP32, tag=f"h32{j}",
                                     name=f"h32{j}") for j in range(2)]
            nc.vector.memset(hT[0][:], 0.0)
            for b in u_bufs + h_bufs + h32_bufs:
                nc.vector.memset(b[:], 0.0)

            # ---------------- Phase 1: xproj = x @ W_ih.T + (b_ih + b_hh) ----
            with (
                tc.tile_pool(name="p1sb", bufs=3) as p1sb,
                tc.tile_pool(name="p1w", bufs=1) as p1w,
                tc.tile_pool(name="p1ps", bufs=2, space="PSUM") as p1ps,
            ):
                wihT = p1w.tile([128, KT, D], FP16)
                bb = p1w.tile([128, D], FP16)
                nc.sync.dma_start(whhT[:], whhT_in[:])
                nc.sync.dma_start(wihT[:], wihT_in[:])
                # ident/bb funnel through a DVE copy so the K-accumulation's
                # first matmul (which also pays the PSUM-release wait) depends
                # on the single DVE semaphore, not per-queue DMA sems.
                with tc.tile_pool(name="stage", bufs=1) as stage_pool:
                    for dst, src, tg in ((ident, id_in, "sm"), (bb, bb_in, "sb"),
                                         (identh, idh_in, "sh")):
                        stage = stage_pool.tile(list(dst.shape), dst.dtype,
                                                tag=tg, name="stage")
                        nc.sync.dma_start(stage[:], src[:])
                        nc.vector.tensor_copy(dst[:], stage[:])

                for blk in range(nblk):
                    xt = p1sb.tile([128, KT, 128], FP16, tag="xt")
                    nc.sync.dma_start(xt[:],
                                      xT_in[blk].rearrange("p (k f) -> p k f", k=KT))
                    xp = p1sb.tile([128, D], FP16, tag="xp")
                    for h in range(2):
                        psum = p1ps.tile([128, 512], FP32, tag=f"ps{h}")
                        cols = slice(512 * h, 512 * h + 512)
                        nc.tensor.matmul(psum[:], identh[:], bb[:, cols],
                                         start=True, stop=False)
                        for k in range(KT):
                            nc.tensor.matmul(psum[:], xt[:, k, :], wihT[:, k, cols],
                                             start=False, stop=(k == KT - 1))
                        nc.vector.tensor_copy(xp[:, cols], psum[:])
                    nc.sync.dma_start(xp_dram[blk * 128:(blk + 1) * 128, :], xp[:])

            # ---------------- Phase 2: sequential recurrence ----------------
            with tc.tile_pool(name="p2ps", bufs=2, space="PSUM") as p2ps:
                n_iter = t_total // U

                def step_body(iv, u):
                    src = hT[u % 2]
                    dst = hT[1 - (u % 2)]
                    row0 = iv * (U * B_LOC) + u * B_LOC

                    u_t = u_bufs[u % 4]
                    # gpsimd (SWDGE) for the dynamic load: keeps the SP
                    # engine's bounds-check register budget for the y store.
                    nc.gpsimd.dma_start(u_t[0:B_LOC, :], xp_dram[ds(row0, B_LOC), :])

                    ps = [p2ps.tile([B_LOC, 512], FP32, tag=f"pre{h}",
                                    name=f"pre{h}") for h in range(2)]
                    for h in range(2):
                        cols = slice(512 * h, 512 * h + 512)
                        nc.tensor.matmul(ps[h][:], identh[:, 0:B_LOC], u_t[:, cols],
                                         start=True, stop=False)
                        for c in range(KT):
                            nc.tensor.matmul(ps[h][:],
                                             src[:, c * B_LOC:(c + 1) * B_LOC],
                                             whhT[:, c, cols],
                                             start=False, stop=(c == KT - 1))

                    h_sb = h_bufs[u % 2]
                    h32 = h32_bufs[u % 2]
                    for h in range(2):
                        cols = slice(512 * h, 512 * h + 512)
                        nc.scalar.activation(h_sb[0:B_LOC, cols], ps[h][:],
                                             mybir.ActivationFunctionType.Tanh)
                    # y wants fp32; the up-cast runs off the critical path
                    nc.vector.tensor_copy(h32[:], h_sb[0:B_LOC, :])
                    nc.sync.dma_start(y_out[ds(row0, B_LOC), :], h32[:])

                    for c in range(KT):
                        tp = p2ps.tile([128, 128], FP16, tag="tp")
                        nc.tensor.transpose(tp[:], h_sb[:, c * 128:(c + 1) * 128],
                                            identh[:])
                        nc.vector.tensor_copy(dst[:, c * B_LOC:(c + 1) * B_LOC],
                                              tp[:, 0:B_LOC])

                with tc.For_i(0, n_iter, 1, staggered_reset=True,
                              hint_engines=(mybir.EngineType.PE,)) as iv:
                    for u in range(U):
                        step_body(iv, u)

    nc.compile()
    return nc


def host_prep(x_shard, t_total=T_TOTAL):
    """Build the per-core input map from a [t, B_LOC, D] x shard."""
    nblk = t_total * B_LOC // 128
    xr = np.ascontiguousarray(x_shard).reshape(nblk, TBLK, B_LOC, KT, 128)
    # -> [blk, p, k, tl, b] -> [blk, 128, KT*128]
    xT = np.ascontiguousarray(
        xr.transpose(0, 4, 3, 1, 2).astype(np.float16)
    ).reshape(nblk, 128, KT * 128)
    return {"xT": xT}


def _shared_inputs(W_ih, W_hh, b_ih, b_hh):
    wihT = np.ascontiguousarray(W_ih.T).reshape(KT, 128, D).transpose(1, 0, 2)
    whhT = np.ascontiguousarray(W_hh.T).reshape(KT, 128, D).transpose(1, 0, 2)
    bb = np.tile((b_ih + b_hh).astype(np.float16)[None, :], (128, 1))
    return {
        "wihT": np.ascontiguousarray(wihT, dtype=np.float16),
        "whhT": np.ascontiguousarray(whhT, dtype=np.float16),
        "bb": bb,
        "ident": np.eye(128, dtype=np.float32),
        "identh": np.eye(128, dtype=np.float16),
    }


_NC_CACHE = {}


def _get_program(t_total):
    if t_total not in _NC_CACHE:
        _NC_CACHE[t_total] = build_program(t_total)
    return _NC_CACHE[t_total]


def kernel(x, W_ih, W_hh, b_ih, b_hh, _t_total=None, _trace=False):
    x = np.asarray(x, dtype=np.float32)
    W_ih = np.asarray(W_ih, dtype=np.float32)
    W_hh = np.asarray(W_hh, dtype=np.float32)
    b_ih = np.asarray(b_ih, dtype=np.float32)
    b_hh = np.asarray(b_hh, dtype=np.float32)

    t_total = _t_total or x.shape[0]
    nc = _get_program(t_total)
    shared = _shared_inputs(W_ih, W_hh, b_ih, b_hh)

    in_maps = []
    for c in range(N_CORES):
        shard = x[:, c * B_LOC:(c + 1) * B_LOC, :]
        m = dict(shared)
        m.update(host_prep(shard, t_total))
        in_maps.append(m)

    res = run_bass_kernel_spmd(nc, in_maps, list(range(N_CORES)), trace=_trace)
    ys = [res.results[c]["y"].reshape(t_total, B_LOC, D) for c in range(N_CORES)]
    y = np.concatenate(ys, axis=1)
    out = (y, y[-1].copy())
    if _trace:
        return out, res
    return out


# revision 26
# speedup vs baseline: 1.2509x; 1.0428x over previous
"""CustomRNN (Elman cell) Trainium2 kernel.

Problem: x [T=512, B=64, D=1024], W_ih/W_hh [1024,1024], b_ih/b_hh [1024].
  xproj = einsum('tbd,hd->tbh', x, W_ih) + b_ih
  h_t = tanh(xproj[t] + h_{t-1} @ W_hh.T + b_hh),  h_0 = 0
  returns (output [T,B,D] = all h_t, h_final [B,D])

Strategy: data-parallel over batch B across 8 cores (8 samples/core).
Each core:
  Phase 1: xproj GEMM (with both biases folded in via an identity-matmul
           accumulation) -> DRAM scratch, natural [t*8+b, d] layout.
  Phase 2: 512 sequential steps. State kept transposed (hT [128, 64] in
           SBUF, chunk c at cols 8c:8c+8 holding h[b, 128c+p]).
           Per step: pre = u_t (identity matmul) + sum_c hT_c.T @ W_hhT_c
           accumulated in PSUM [8, 1024] (2 banks), tanh on ACT -> h [8,1024],
           DMA h to y, PE-transpose h back to hT for the next step.
All matmuls use K=128 contraction. Weights pre-transposed on host.

Walrus constraint baked into the structure: one sync wait per DMA
instruction, two per compute instruction. One-time loads funnel through a
DVE copy; persistent buffers are allocated before phase 1 so no SBUF
address reuse creates cross-engine WAR deps.
"""

import os
import sys

import numpy as np

for _p in ("/opt/trn_rl_repo", "/root/.axon_site/_ro/trn_rl_repo"):
    if os.path.isdir(_p) and _p not in sys.path:
        sys.path.append(_p)

import concourse.bass as bass
import concourse.mybir as mybir
import concourse.tile as tile
from concourse import bacc
from concourse.bass import ds
from concourse.bass_utils import run_bass_kernel_spmd

T_TOTAL = 512
B_TOTAL = 64
D = 1024
N_CORES = 8
B_LOC = B_TOTAL // N_CORES          # 8 samples per core
KT = D // 128                       # 8 contraction chunks
TBLK = 16                           # timesteps per xproj tile (16*8 = 128 rows)
U = 32                              # unrolled steps per For_i iteration

FP32 = mybir.dt.float32
FP16 = mybir.dt.float16


def build_program(t_total=T_TOTAL):
    nblk = t_total * B_LOC // 128
    nc = bacc.Bacc("TRN2", target_bir_lowering=False, debug=False)

    xT_in = nc.declare_dram_parameter("xT", [nblk, 128, KT * 128], FP16, isOutput=False)
    wihT_in = nc.declare_dram_parameter("wihT", [128, KT, D], FP16, isOutput=False)
    whhT_in = nc.declare_dram_parameter("whhT", [128, KT, D], FP16, isOutput=False)
    bb_in = nc.declare_dram_parameter("bb", [128, D], FP16, isOutput=False)
    id_in = nc.declare_dram_parameter("ident", [128, 128], FP32, isOutput=False)
    idh_in = nc.declare_dram_parameter("identh", [128, 128], FP16, isOutput=False)
    y_out = nc.declare_dram_parameter("y", [t_total * B_LOC, D], FP32, isOutput=True)

    xp_dram = nc.dram_tensor("xp_scratch", [t_total * B_LOC, D], FP16)

    with tile.TileContext(nc) as tc:
        with tc.tile_pool(name="persist", bufs=1) as persist:
            ident = persist.tile([128, 128], FP32)
            identh = persist.tile([128, 128], FP16)
            whhT = persist.tile([128, KT, D], FP16)
            hT = [persist.tile([128, B_LOC * KT], FP16, tag=f"hT{i}",
                               name=f"hT{i}") for i in range(2)]
            # u/h buffers are persistent and rotated manually so the
            # never-DMA'd rows 8..127 (read by K=128 matmuls against
            # zero lanes of the selection matrix) are memset exactly once.
            u_bufs = [persist.tile([128, D], FP16, tag=f"u{j}",
                                   name=f"u{j}") for j in range(4)]
            h_bufs = [persist.tile([128, D], FP16, tag=f"hb{j}",
                                   name=f"hb{j}") for j in range(2)]
            h32_bufs = [persist.tile([B_LOC, D], FP32, tag=f"h32{j}",
                                     name=f"h32{j}") for j in range(2)]
            nc.vector.memset(hT[0][:], 0.0)
            for b in u_bufs + h_bufs + h32_bufs:
                nc.vector.memset(b[:], 0.0)

            # ---------------- Phase 1: xproj = x @ W_ih.T + (b_ih + b_hh) ----
            with (
                tc.tile_pool(name="p1sb", bufs=3) as p1sb,
                tc.tile_pool(name="p1w", bufs=1) as p1w,
                tc.tile_pool(name="p1ps", bufs=2, space="PSUM") as p1ps,
            ):
                wihT = p1w.tile([128, KT, D], FP16)
                bb = p1w.tile([128, D], FP16)
                nc.sync.dma_start(whhT[:], whhT_in[:])
                nc.sync.dma_start(wihT[:], wihT_in[:])
                # ident/bb funnel through a DVE copy so the K-accumulation's
                # first matmul (which also pays the PSUM-release wait) depends
                # on the single DVE semaphore, not per-queue DMA sems.
                with tc.tile_pool(name="stage", bufs=1) as stage_pool:
                    for dst, src, tg in ((ident, id_in, "sm"), (bb, bb_in, "sb"),
                                         (identh, idh_in, "sh")):
                        stage = stage_pool.tile(list(dst.shape), dst.dtype,
                                                tag=tg, name="stage")
                        nc.sync.dma_start(stage[:], src[:])
                        nc.vector.tensor_copy(dst[:], stage[:])

                for blk in range(nblk):
                    xt = p1sb.tile([128, KT, 128], FP16, tag="xt")
                    nc.sync.dma_start(xt[:],
                                      xT_in[blk].rearrange("p (k f) -> p k f", k=KT))
                    xp = p1sb.tile([128, D], FP16, tag="xp")
                    for h in range(2):
                        psum = p1ps.tile([128, 512], FP32, tag=f"ps{h}")
                        cols = slice(512 * h, 512 * h + 512)
                        nc.tensor.matmul(psum[:], identh[:], bb[:, cols],
                                         start=True, stop=False)
                        for k in range(KT):
                            nc.tensor.matmul(psum[:], xt[:, k, :], wihT[:, k, cols],
                                             start=False, stop=(k == KT - 1))
                        nc.vector.tensor_copy(xp[:, cols], psum[:])
                    nc.sync.dma_start(xp_dram[blk * 128:(blk + 1) * 128, :], xp[:])

            # ---------------- Phase 2: sequential recurrence ----------------
            with tc.tile_pool(name="p2ps", bufs=2, space="PSUM") as p2ps:
                n_iter = t_total // U

                def step_body(iv, u):
                    src = hT[u % 2]
                    dst = hT[1 - (u % 2)]
                    row0 = iv * (U * B_LOC) + u * B_LOC

                    u_t = u_bufs[u % 4]
                    # gpsimd (SWDGE) for the dynamic load: keeps the SP
                    # engine's bounds-check register budget for the y store.
                    nc.gpsimd.dma_start(u_t[0:B_LOC, :], xp_dram[ds(row0, B_LOC), :])

                    ps = [p2ps.tile([B_LOC, 512], FP32, tag=f"pre{h}",
                                    name=f"pre{h}") for h in range(2)]
                    for h in range(2):
                        cols = slice(512 * h, 512 * h + 512)
                        nc.tensor.matmul(ps[h][:], identh[:, 0:B_LOC], u_t[:, cols],
                                         start=True, stop=False)
                        for c in range(KT):
                            nc.tensor.matmul(ps[h][:],
                                             src[:, c * B_LOC:(c + 1) * B_LOC],
                                             whhT[:, c, cols],
                                             start=False, stop=(c == KT - 1))

                    h_sb = h_bufs[u % 2]
                    h32 = h32_bufs[u % 2]
                    for h in range(2):
                        cols = slice(512 * h, 512 * h + 512)
                        nc.scalar.activation(h_sb[0:B_LOC, cols], ps[h][:],
                                             mybir.ActivationFunctionType.Tanh)
                    # y wants fp32; the up-cast runs off the critical path
                    nc.vector.tensor_copy(h32[:], h_sb[0:B_LOC, :])
                    nc.sync.dma_start(y_out[ds(row0, B_LOC), :], h32[:])

                    for c in range(KT):
                        tp = p2ps.tile([128, 128], FP16, tag="tp")
                        nc.tensor.transpose(tp[:], h_sb[:, c * 128:(c + 1) * 128],
                                            identh[:])
                        nc.vector.tensor_copy(dst[:, c * B_LOC:(c + 1) * B_LOC],
                                              tp[:, 0:B_LOC])

                with tc.For_i(0, n_iter, 1, staggered_reset=True,
                              hint_engines=(mybir.EngineType.PE,)) as iv:
                    for u in range(U):
                        step_body(iv, u)

    nc.compile()
    return nc


def host_prep(x_shard, t_total=T_TOTAL):
    """Build the per-core input map from a [t, B_LOC, D] x shard."""
    nblk = t_total * B_LOC // 128
    xr = np.ascontiguousarray(x_shard).reshape(nblk, TBLK, B_LOC, KT, 128)
    # -> [blk, p, k, tl, b] -> [blk, 128, KT*128]
    xT = np.ascontiguousarray(
        xr.transpose(0, 4, 3, 1, 2).astype(np.float16)
    ).reshape(nblk, 128, KT * 128)
    return {"xT": xT}


def _shared_inputs(W_ih, W_hh, b_ih, b_hh):
    wihT = np.ascontiguousarray(W_ih.T).reshape(KT, 128, D).transpose(1, 0, 2)
    whhT = np.ascontiguousarray(W_hh.T).reshape(KT, 128, D).transpose(1, 0, 2)
    bb = np.tile((b_ih + b_hh).astype(np.float16)[None, :], (128, 1))
    return {
        "wihT": np.ascontiguousarray(wihT, dtype=np.float16),
        "whhT": np.ascontiguousarray(whhT, dtype=np.float16),
        "bb": bb,
        "ident": np.eye(128, dtype=np.float32),
        "identh": np.eye(128, dtype=np.float16),
    }


_NC_CACHE = {}


def _get_program(t_total):
    if t_total not in _NC_CACHE:
        _NC_CACHE[t_total] = build_program(t_total)
    return _NC_CACHE[t_total]


def kernel(x, W_ih, W_hh, b_ih, b_hh, _t_total=None, _trace=False):
    x = np.asarray(x, dtype=np.float32)
    W_ih = np.asarray(W_ih, dtype=np.float32)
    W_hh = np.asarray(W_hh, dtype=np.float32)
    b_ih = np.asarray(b_ih, dtype=np.float32)
    b_hh = np.asarray(b_hh, dtype=np.float32)

    t_total = _t_total or x.shape[0]
    nc = _get_program(t_total)
    shared = _shared_inputs(W_ih, W_hh, b_ih, b_hh)

    in_maps = []
    for c in range(N_CORES):
        shard = x[:, c * B_LOC:(c + 1) * B_LOC, :]
        m = dict(shared)
        m.update(host_prep(shard, t_total))
        in_maps.append(m)

    res = run_bass_kernel_spmd(nc, in_maps, list(range(N_CORES)), trace=_trace)
    ys = [res.results[c]["y"].reshape(t_total, B_LOC, D) for c in range(N_CORES)]
    y = np.concatenate(ys, axis=1)
    out = (y, y[-1].copy())
    if _trace:
        return out, res
    return out
